# revision 43
# baseline (speedup 1.0000x reference)
"""Trainium2 Bass kernel for nn_Net_time_no_conv (single-timestep SNN forward).

Math: the first layer binarizes data_in[B,2] elementwise (spk = (x > th0)),
so every downstream value is a function of just 2 bits per row.  The whole
network therefore emits one of only 4 possible (spk3, mem3) row pairs.
We compute the 4-entry LUT on host (float64, from the passed-in weights),
then the device kernel per row computes
    a = (x0 > t0), b = (x1 > t1)            in {0,1}
    mem_j = c0_j + ca_j*a + cb_j*b + cab_j*a*b   (bilinear expansion)
    spk_j = (mem_j > thr)
which is exact (the c's are derived from the LUT).

Sharding: pure data parallel — batch split evenly over 8 cores, no
communication.  Per core: 32768 rows; read 256KB, write 1.5MB.
"""

import os
import numpy as np

N_CORES = 8
B_TOTAL = 262144
SHARD = B_TOTAL // N_CORES      # 32768 rows per core
P = 128                          # SBUF partitions

# --- tunables -------------------------------------------------------------
# rows-per-partition per chunk; must sum to SHARD // P = 256.
# small first chunk: its input DMA lands sooner, compute starts earlier.
CHUNKS = [int(c) for c in os.environ.get("K_CHUNKS", "48,160,48").split(",")]
SPK_ENG = os.environ.get("K_SPK", "v")               # spike compare engine
CMPB_ENG = os.environ.get("K_CMPB", "v")             # (x1>t1) compare engine
BUFS = int(os.environ.get("K_BUFS", "3"))            # tile pool slots per tag
PART_ID = bool(int(os.environ.get("K_PARTID", "0")))  # emit partition-id preamble
# --------------------------------------------------------------------------

LAST_RESULT = None  # BassKernelResults of the most recent run (for test.py)


def _host_lut(inputs):
    """Forward the 4 possible binarized inputs through the net in float64.

    Returns (spk_lut[2,2,6], mem_lut[2,2,6], margins dict) where index [a,b]
    corresponds to a = (x0 > th0[0]), b = (x1 > th0[1]).
    """
    f8 = np.float64
    W = [np.asarray(inputs[f"W{i}"], f8) for i in range(4)]
    b = [np.asarray(inputs[f"b{i}"], f8) for i in range(4)]
    th = [float(np.asarray(inputs[f"th{i}"])) for i in range(1, 5)]

    # s rows: (a,b) = (0,0),(1,0),(0,1),(1,1)
    s = np.array([[0., 0.], [1., 0.], [0., 1.], [1., 1.]], f8)
    margins = []
    cur = s @ W[0].T + b[0]
    margins.append(np.min(np.abs(cur - th[0])))
    spk = (cur > th[0]).astype(f8)
    cur = spk @ W[1].T + b[1]
    margins.append(np.min(np.abs(cur - th[1])))
    spk = (cur > th[1]).astype(f8)
    cur = spk @ W[2].T + b[2]
    margins.append(np.min(np.abs(cur - th[2])))
    spk = (cur > th[2]).astype(f8)
    cur3 = spk @ W[3].T + b[3]                       # [4, 6] = mem3 candidates
    final_margin = np.min(np.abs(cur3 - th[3]))
    spk3 = (cur3 > th[3]).astype(f8)

    # s rows are [(0,0),(1,0),(0,1),(1,1)] = index a + 2b
    mem_lut = np.zeros((2, 2, 6), f8)
    spk_lut = np.zeros((2, 2, 6), f8)
    for idx, (a, bb) in enumerate([(0, 0), (1, 0), (0, 1), (1, 1)]):
        mem_lut[a, bb] = cur3[idx]
        spk_lut[a, bb] = spk3[idx]
    return spk_lut, mem_lut, dict(hidden_margins=margins, final_margin=final_margin,
                                  cur3=cur3, th_final=th[3])


def _bilinear_coeffs(lut):
    """lut[a,b,j] -> c0, ca, cb, cab  (each [6] float64)."""
    c0 = lut[0, 0]
    ca = lut[1, 0] - lut[0, 0]
    cb = lut[0, 1] - lut[0, 0]
    cab = lut[1, 1] - lut[1, 0] - lut[0, 1] + lut[0, 0]
    return c0, ca, cb, cab


def _build_nc(t0, t1, c0, ca, cb, cab, spk_thr):
    """Build the Bass/Tile program (same program for every core).

    t0, t1: input thresholds (floats).
    c0..cab: [6] float arrays, bilinear coefficients of mem3.
    spk_thr: either float (uniform threshold) or [6] array (per-column).
    """
    import concourse.mybir as mybir
    from concourse import bacc
    from concourse.tile import TileContext

    f32 = mybir.dt.float32
    Alu = mybir.AluOpType
    Act = mybir.ActivationFunctionType

    assert sum(CHUNKS) * P == SHARD, CHUNKS
    starts = np.cumsum([0] + CHUNKS[:-1])

    nc = bacc.Bacc(None, target_bir_lowering=False,
                   enable_partition_id=PART_ID)
    u8 = mybir.dt.uint8
    x = nc.dram_tensor("x", [SHARD, 2], f32, kind="ExternalInput")
    # spikes are exactly 0/1 -> ship as uint8 (4x fewer output bytes);
    # the host converts back to float32 (exact)
    spk_o = nc.dram_tensor("spk", [SHARD, 6], u8, kind="ExternalOutput")
    mem_o = nc.dram_tensor("mem", [SHARD, 6], f32, kind="ExternalOutput")

    # partition p owns rows [p*256, (p+1)*256); chunk c covers rows
    # [starts[c], starts[c]+R) within each partition's range.
    RT = SHARD // P                       # 256 rows per partition total
    x_pr = x[:].rearrange("(p r) i -> p (r i)", p=P, r=RT)
    spk_pr = spk_o[:].rearrange("(p r) j -> p (r j)", p=P, r=RT)
    mem_pr = mem_o[:].rearrange("(p r) j -> p (r j)", p=P, r=RT)

    per_col_thr = not np.isscalar(spk_thr)

    with TileContext(nc) as tc:
        with tc.tile_pool(name="pool", bufs=BUFS) as pool:
            tau = None
            if per_col_thr:
                tau = pool.tile([P, 6], f32, tag="tau")
                for j in range(6):
                    nc.vector.memset(tau[:, j:j + 1], float(spk_thr[j]))

            # per-partition bias constants for ScalarE activations
            # layout: u biases at [0:6], v biases at [6:12]
            biases = [float(x) for x in cb] + [float(x) for x in c0]
            bias_t = pool.tile([P, 12], f32, tag="bias")
            for i, bv in enumerate(biases):
                nc.vector.memset(bias_t[:, i:i + 1], bv)

            engines = {"s": nc.scalar, "v": nc.vector}

            def narrow_op(asgn, dst, scale, bias, bias_idx, src):
                if asgn == "s":
                    engines["s"].activation(dst, src, Act.Identity,
                                            bias=bias_t[:, bias_idx:bias_idx + 1],
                                            scale=float(scale))
                else:
                    engines[asgn].tensor_scalar(dst, src, float(scale), float(bias),
                                                Alu.mult, Alu.add)

            for ci, (R, r0) in enumerate(zip(CHUNKS, starts)):
                # per-chunk scalar/vector narrow split: minimize
                # max(vector busy, scalar busy) for this chunk size
                v_cost = (58 + R) / 0.96          # strided-dst TS
                s_cost = (R + 352) / 1.2          # ACTIVATE
                v_base = (2 * (58 + R) + 2 * (151 + 6 * R) + (58 + 3 * R)) / 0.96
                best_nv = min(range(0, 13), key=lambda nv: max(
                    v_base + nv * v_cost, (12 - nv) * s_cost))
                asgn = "v" * best_nv + "s" * (12 - best_nv)
                u_asgn, v_asgn = asgn[0::2], asgn[1::2]

                xt = pool.tile([P, 2 * R], f32, tag="x")
                nc.sync.dma_start(out=xt[:], in_=x_pr[:, 2 * r0:2 * (r0 + R)])
                x3 = xt[:].rearrange("p (r i) -> p r i", i=2)

                a = pool.tile([P, R], f32, tag="a")
                bt = pool.tile([P, R], f32, tag="b")
                nc.vector.tensor_scalar(a[:], x3[:, :, 0], float(t0), None, Alu.is_gt)
                engines[CMPB_ENG].tensor_scalar(bt[:], x3[:, :, 1], float(t1), None,
                                                Alu.is_gt)

                # u6/v6 (r, j)-interleaved: narrows write strided (1x) but the
                # wide TTs read contiguously at full rate (the winning tradeoff)
                u6 = pool.tile([P, 6 * R], f32, tag="u6")
                v6 = pool.tile([P, 6 * R], f32, tag="v6")
                u_rj = u6[:].rearrange("p (r j) -> p r j", j=6)
                v_rj = v6[:].rearrange("p (r j) -> p r j", j=6)

                # u_j = cab_j*a + cb_j  (emitted first: they gate the wide mult)
                for j in range(6):
                    narrow_op(u_asgn[j], u_rj[:, :, j], cab[j], cb[j], j, a[:])

                w6 = pool.tile([P, 6 * R], f32, tag="w6")
                w3 = w6[:].rearrange("p (r j) -> p r j", j=6)
                b_rep = bt[:, :, None].to_broadcast([P, R, 6])
                nc.vector.tensor_tensor(out=w3, in0=b_rep, in1=u_rj, op=Alu.mult)

                # v_j = ca_j*a + c0_j  (overlap the wide mult)
                for j in range(6):
                    narrow_op(v_asgn[j], v_rj[:, :, j], ca[j], c0[j], 6 + j, a[:])

                m6 = pool.tile([P, 6 * R], f32, tag="m6")
                m3 = m6[:].rearrange("p (r j) -> p r j", j=6)
                nc.vector.tensor_tensor(out=m3, in0=v_rj, in1=w3, op=Alu.add)
                nc.sync.dma_start(out=mem_pr[:, 6 * r0:6 * (r0 + R)], in_=m6[:])

                s6 = pool.tile([P, 6 * R], u8, tag="s6")
                if per_col_thr:
                    s3 = s6[:].rearrange("p (r j) -> p r j", j=6)
                    tau_rep = tau[:, None, :].to_broadcast([P, R, 6])
                    engines[SPK_ENG].tensor_tensor(out=s3, in0=m3, in1=tau_rep,
                                                   op=Alu.is_gt)
                else:
                    engines[SPK_ENG].tensor_scalar(s6[:], m6[:], float(spk_thr), None,
                                                   Alu.is_gt)
                nc.sync.dma_start(out=spk_pr[:, 6 * r0:6 * (r0 + R)], in_=s6[:])
    return nc


def _build_const_nc(mem_row, spk_row):
    """Fast path: the 4 LUT rows are identical, so the output is constant.

    No input-dependent compute at all: fill SBUF with the constant rows and
    stream them out.  Pure output-bandwidth kernel.
    mem_row: [6] float32 values; spk_row: [6] ints (0/1).
    """
    import concourse.mybir as mybir
    from concourse import bacc
    from concourse.tile import TileContext

    f32 = mybir.dt.float32
    u8 = mybir.dt.uint8

    nc = bacc.Bacc(None, target_bir_lowering=False,
                   enable_partition_id=PART_ID)
    x = nc.dram_tensor("x", [SHARD, 2], f32, kind="ExternalInput")
    spk_o = nc.dram_tensor("spk", [SHARD, 6], u8, kind="ExternalOutput")
    mem_o = nc.dram_tensor("mem", [SHARD, 6], f32, kind="ExternalOutput")

    RT = SHARD // P                       # 256 rows per partition
    x_pr = x[:].rearrange("(p r) i -> p (r i)", p=P, r=RT)
    spk_pr = spk_o[:].rearrange("(p r) j -> p (r j)", p=P, r=RT)
    mem_pr = mem_o[:].rearrange("(p r) j -> p (r j)", p=P, r=RT)

    # pieces sized so every DMA descriptor is >=1.5KB per partition (small
    # descriptors measurably drop queue bandwidth); output split across the
    # two DMA queues (sync HWDGE ring: p0+p2, gpsimd SWDGE ring: spk+p1)
    PIECES = [96, 64, 96]
    assert sum(PIECES) == RT
    p_starts = np.cumsum([0] + PIECES[:-1])
    piece_q = ["sync", "gpsimd", "sync"]

    del x_pr  # input is declared (keeps the NEFF binding) but never read

    with TileContext(nc) as tc:
        with tc.tile_pool(name="pool", bufs=BUFS) as pool:
            # spikes first: the gpsimd (SWDGE) queue has ~2us to first byte,
            # so its DMA must issue as early as possible.  Uniform row fills
            # via a uint32 view (4 bytes/elem); otherwise per-column strided.
            s6 = pool.tile([P, 6 * RT], u8, tag="s6")
            svals = [int(v) for v in spk_row]
            if len(set(svals)) == 1:
                s6_u32 = s6[:].bitcast(mybir.dt.uint32)
                nc.vector.memset(s6_u32, svals[0] * 0x01010101)
            else:
                s3 = s6[:].rearrange("p (r j) -> p r j", j=6)
                for j in range(6):
                    nc.vector.memset(s3[:, :, j], svals[j])
            nc.gpsimd.dma_start(out=spk_pr[:], in_=s6[:])

            # one [P, 6] tile holding the constant mem row
            mc = pool.tile([P, 6], f32, tag="mc")
            for j in range(6):
                nc.vector.memset(mc[:, j:j + 1], float(mem_row[j]))

            # whole-shard mem tile: pieces are views, so no pool-slot
            # recycling ever gates a fill on an earlier DMA's completion
            m6 = pool.tile([P, 6 * RT], f32, tag="m6")
            for R, r0, q in zip(PIECES, p_starts, piece_q):
                mp = m6[:, 6 * r0:6 * (r0 + R)]
                m3 = mp.rearrange("p (r j) -> p r j", j=6)
                nc.vector.tensor_copy(
                    out=m3, in_=mc[:, None, :].to_broadcast([P, R, 6]))
                eng = nc.sync if q == "sync" else nc.gpsimd
                eng.dma_start(out=mem_pr[:, 6 * r0:6 * (r0 + R)], in_=mp)
    return nc


def _spk_threshold(info):
    """Pick the device-side spike threshold.

    If every LUT mem3 value is comfortably far from th_final, a uniform
    immediate threshold is safe (and fastest).  Otherwise use per-column
    midpoint thresholds, which are robust to any fp32 rounding.
    """
    if info["final_margin"] > 1e-3:
        return float(info["th_final"])
    cur3 = info["cur3"]          # [4, 6] float64
    th = info["th_final"]
    tau = np.empty(6)
    for j in range(6):
        vals = cur3[:, j]
        lo = vals[vals <= th]
        hi = vals[vals > th]
        if len(hi) == 0:
            tau[j] = 1e30
        elif len(lo) == 0:
            tau[j] = -1e30
        else:
            tau[j] = (lo.max() + hi.min()) / 2.0
    return tau


def _ensure_ntff_hook():
    """Some images lack antenv.axon_hooks; if BASS_TRACE is set without it,
    run_bass_kernel_spmd would crash on import.  Provide a working shim."""
    try:
        import antenv.axon_hooks  # noqa: F401
        return
    except ImportError:
        pass
    try:
        import sys
        import types
        import antenv
        hook = None
        try:
            from trn_agent_boot.trn_boot import _ntff_profile_via_ctypes
            hook = _ntff_profile_via_ctypes("/opt/axon/libaxon_pjrt.so")
        except Exception:
            hook = None
        mod = types.ModuleType("antenv.axon_hooks")
        mod.get_axon_ntff_profile_hook = lambda: hook
        mod.set_axon_ntff_profile_hook = lambda h: None
        sys.modules["antenv.axon_hooks"] = mod
        antenv.axon_hooks = mod
    except Exception:
        pass


def kernel(**inputs):
    global LAST_RESULT
    _ensure_ntff_hook()
    from concourse.bass_utils import run_bass_kernel_spmd

    data_in = np.ascontiguousarray(np.asarray(inputs["data_in"], np.float32))
    assert data_in.shape == (B_TOTAL, 2)

    spk_lut, mem_lut, info = _host_lut(inputs)
    c0, ca, cb, cab = _bilinear_coeffs(mem_lut)
    th0 = np.asarray(inputs["th0"], np.float64)
    spk_thr = _spk_threshold(info)

    # Fast path: if the 4 LUT rows coincide exactly (spikes die out in the
    # hidden layers) and every threshold margin is comfortable, the output is
    # a constant independent of data_in.
    cur3 = info["cur3"]
    const_out = (bool(np.all(cur3 == cur3[0]))
                 and min(info["hidden_margins"]) > 1e-4
                 and info["final_margin"] > 1e-3)
    if const_out:
        nc = _build_const_nc(mem_lut[0, 0].astype(np.float32),
                             spk_lut[0, 0].astype(np.int64))
    else:
        nc = _build_nc(float(th0[0]), float(th0[1]), c0, ca, cb, cab, spk_thr)
    nc.compile()

    shards = data_in.reshape(N_CORES, SHARD, 2)
    in_maps = [{"x": shards[i]} for i in range(N_CORES)]
    res = run_bass_kernel_spmd(nc, in_maps, core_ids=list(range(N_CORES)))
    LAST_RESULT = res

    spk = np.concatenate([np.asarray(r["spk"]) for r in res.results], axis=0)
    mem = np.concatenate([np.asarray(r["mem"]) for r in res.results], axis=0)
    return spk.astype(np.float32), mem.astype(np.float32, copy=False)


# revision 47
# speedup vs baseline: 1.1403x; 1.1403x over previous
"""Trainium2 Bass kernel for nn_Net_time_no_conv (single-timestep SNN forward).

Math: the first layer binarizes data_in[B,2] elementwise (spk = (x > th0)),
so every downstream value is a function of just 2 bits per row.  The whole
network therefore emits one of only 4 possible (spk3, mem3) row pairs.
We compute the 4-entry LUT on host (float64, from the passed-in weights),
then the device kernel per row computes
    a = (x0 > t0), b = (x1 > t1)            in {0,1}
    mem_j = c0_j + ca_j*a + cb_j*b + cab_j*a*b   (bilinear expansion)
    spk_j = (mem_j > thr)
which is exact (the c's are derived from the LUT).

Sharding: pure data parallel — batch split evenly over 8 cores, no
communication.  Per core: 32768 rows; read 256KB, write 1.5MB.
"""

import os
import numpy as np

N_CORES = 8
B_TOTAL = 262144
SHARD = B_TOTAL // N_CORES      # 32768 rows per core
P = 128                          # SBUF partitions

# --- tunables -------------------------------------------------------------
# rows-per-partition per chunk; must sum to SHARD // P = 256.
# small first chunk: its input DMA lands sooner, compute starts earlier.
CHUNKS = [int(c) for c in os.environ.get("K_CHUNKS", "48,160,48").split(",")]
SPK_ENG = os.environ.get("K_SPK", "v")               # spike compare engine
CMPB_ENG = os.environ.get("K_CMPB", "v")             # (x1>t1) compare engine
BUFS = int(os.environ.get("K_BUFS", "3"))            # tile pool slots per tag
PART_ID = bool(int(os.environ.get("K_PARTID", "0")))  # emit partition-id preamble
# --------------------------------------------------------------------------

LAST_RESULT = None  # BassKernelResults of the most recent run (for test.py)


def _host_lut(inputs):
    """Forward the 4 possible binarized inputs through the net in float64.

    Returns (spk_lut[2,2,6], mem_lut[2,2,6], margins dict) where index [a,b]
    corresponds to a = (x0 > th0[0]), b = (x1 > th0[1]).
    """
    f8 = np.float64
    W = [np.asarray(inputs[f"W{i}"], f8) for i in range(4)]
    b = [np.asarray(inputs[f"b{i}"], f8) for i in range(4)]
    th = [float(np.asarray(inputs[f"th{i}"])) for i in range(1, 5)]

    # s rows: (a,b) = (0,0),(1,0),(0,1),(1,1)
    s = np.array([[0., 0.], [1., 0.], [0., 1.], [1., 1.]], f8)
    margins = []
    cur = s @ W[0].T + b[0]
    margins.append(np.min(np.abs(cur - th[0])))
    spk = (cur > th[0]).astype(f8)
    cur = spk @ W[1].T + b[1]
    margins.append(np.min(np.abs(cur - th[1])))
    spk = (cur > th[1]).astype(f8)
    cur = spk @ W[2].T + b[2]
    margins.append(np.min(np.abs(cur - th[2])))
    spk = (cur > th[2]).astype(f8)
    cur3 = spk @ W[3].T + b[3]                       # [4, 6] = mem3 candidates
    final_margin = np.min(np.abs(cur3 - th[3]))
    spk3 = (cur3 > th[3]).astype(f8)

    # s rows are [(0,0),(1,0),(0,1),(1,1)] = index a + 2b
    mem_lut = np.zeros((2, 2, 6), f8)
    spk_lut = np.zeros((2, 2, 6), f8)
    for idx, (a, bb) in enumerate([(0, 0), (1, 0), (0, 1), (1, 1)]):
        mem_lut[a, bb] = cur3[idx]
        spk_lut[a, bb] = spk3[idx]
    return spk_lut, mem_lut, dict(hidden_margins=margins, final_margin=final_margin,
                                  cur3=cur3, th_final=th[3])


def _bilinear_coeffs(lut):
    """lut[a,b,j] -> c0, ca, cb, cab  (each [6] float64)."""
    c0 = lut[0, 0]
    ca = lut[1, 0] - lut[0, 0]
    cb = lut[0, 1] - lut[0, 0]
    cab = lut[1, 1] - lut[1, 0] - lut[0, 1] + lut[0, 0]
    return c0, ca, cb, cab


def _build_nc(t0, t1, c0, ca, cb, cab, spk_thr):
    """Build the Bass/Tile program (same program for every core).

    t0, t1: input thresholds (floats).
    c0..cab: [6] float arrays, bilinear coefficients of mem3.
    spk_thr: either float (uniform threshold) or [6] array (per-column).
    """
    import concourse.mybir as mybir
    from concourse import bacc
    from concourse.tile import TileContext

    f32 = mybir.dt.float32
    Alu = mybir.AluOpType
    Act = mybir.ActivationFunctionType

    assert sum(CHUNKS) * P == SHARD, CHUNKS
    starts = np.cumsum([0] + CHUNKS[:-1])

    nc = bacc.Bacc(None, target_bir_lowering=False,
                   enable_partition_id=PART_ID)
    u8 = mybir.dt.uint8
    x = nc.dram_tensor("x", [SHARD, 2], f32, kind="ExternalInput")
    # spikes are exactly 0/1 -> ship as uint8 (4x fewer output bytes);
    # the host converts back to float32 (exact)
    spk_o = nc.dram_tensor("spk", [SHARD, 6], u8, kind="ExternalOutput")
    mem_o = nc.dram_tensor("mem", [SHARD, 6], f32, kind="ExternalOutput")

    # partition p owns rows [p*256, (p+1)*256); chunk c covers rows
    # [starts[c], starts[c]+R) within each partition's range.
    RT = SHARD // P                       # 256 rows per partition total
    x_pr = x[:].rearrange("(p r) i -> p (r i)", p=P, r=RT)
    spk_pr = spk_o[:].rearrange("(p r) j -> p (r j)", p=P, r=RT)
    mem_pr = mem_o[:].rearrange("(p r) j -> p (r j)", p=P, r=RT)

    per_col_thr = not np.isscalar(spk_thr)

    with TileContext(nc) as tc:
        with tc.tile_pool(name="pool", bufs=BUFS) as pool:
            tau = None
            if per_col_thr:
                tau = pool.tile([P, 6], f32, tag="tau")
                for j in range(6):
                    nc.vector.memset(tau[:, j:j + 1], float(spk_thr[j]))

            # per-partition bias constants for ScalarE activations
            # layout: u biases at [0:6], v biases at [6:12]
            biases = [float(x) for x in cb] + [float(x) for x in c0]
            bias_t = pool.tile([P, 12], f32, tag="bias")
            for i, bv in enumerate(biases):
                nc.vector.memset(bias_t[:, i:i + 1], bv)

            engines = {"s": nc.scalar, "v": nc.vector}

            def narrow_op(asgn, dst, scale, bias, bias_idx, src):
                if asgn == "s":
                    engines["s"].activation(dst, src, Act.Identity,
                                            bias=bias_t[:, bias_idx:bias_idx + 1],
                                            scale=float(scale))
                else:
                    engines[asgn].tensor_scalar(dst, src, float(scale), float(bias),
                                                Alu.mult, Alu.add)

            for ci, (R, r0) in enumerate(zip(CHUNKS, starts)):
                # per-chunk scalar/vector narrow split: minimize
                # max(vector busy, scalar busy) for this chunk size
                v_cost = (58 + R) / 0.96          # strided-dst TS
                s_cost = (R + 352) / 1.2          # ACTIVATE
                v_base = (2 * (58 + R) + 2 * (151 + 6 * R) + (58 + 3 * R)) / 0.96
                best_nv = min(range(0, 13), key=lambda nv: max(
                    v_base + nv * v_cost, (12 - nv) * s_cost))
                asgn = "v" * best_nv + "s" * (12 - best_nv)
                u_asgn, v_asgn = asgn[0::2], asgn[1::2]

                xt = pool.tile([P, 2 * R], f32, tag="x")
                nc.sync.dma_start(out=xt[:], in_=x_pr[:, 2 * r0:2 * (r0 + R)])
                x3 = xt[:].rearrange("p (r i) -> p r i", i=2)

                a = pool.tile([P, R], f32, tag="a")
                bt = pool.tile([P, R], f32, tag="b")
                nc.vector.tensor_scalar(a[:], x3[:, :, 0], float(t0), None, Alu.is_gt)
                engines[CMPB_ENG].tensor_scalar(bt[:], x3[:, :, 1], float(t1), None,
                                                Alu.is_gt)

                # u6/v6 (r, j)-interleaved: narrows write strided (1x) but the
                # wide TTs read contiguously at full rate (the winning tradeoff)
                u6 = pool.tile([P, 6 * R], f32, tag="u6")
                v6 = pool.tile([P, 6 * R], f32, tag="v6")
                u_rj = u6[:].rearrange("p (r j) -> p r j", j=6)
                v_rj = v6[:].rearrange("p (r j) -> p r j", j=6)

                # u_j = cab_j*a + cb_j  (emitted first: they gate the wide mult)
                for j in range(6):
                    narrow_op(u_asgn[j], u_rj[:, :, j], cab[j], cb[j], j, a[:])

                w6 = pool.tile([P, 6 * R], f32, tag="w6")
                w3 = w6[:].rearrange("p (r j) -> p r j", j=6)
                b_rep = bt[:, :, None].to_broadcast([P, R, 6])
                nc.vector.tensor_tensor(out=w3, in0=b_rep, in1=u_rj, op=Alu.mult)

                # v_j = ca_j*a + c0_j  (overlap the wide mult)
                for j in range(6):
                    narrow_op(v_asgn[j], v_rj[:, :, j], ca[j], c0[j], 6 + j, a[:])

                m6 = pool.tile([P, 6 * R], f32, tag="m6")
                m3 = m6[:].rearrange("p (r j) -> p r j", j=6)
                nc.vector.tensor_tensor(out=m3, in0=v_rj, in1=w3, op=Alu.add)
                nc.sync.dma_start(out=mem_pr[:, 6 * r0:6 * (r0 + R)], in_=m6[:])

                s6 = pool.tile([P, 6 * R], u8, tag="s6")
                if per_col_thr:
                    s3 = s6[:].rearrange("p (r j) -> p r j", j=6)
                    tau_rep = tau[:, None, :].to_broadcast([P, R, 6])
                    engines[SPK_ENG].tensor_tensor(out=s3, in0=m3, in1=tau_rep,
                                                   op=Alu.is_gt)
                else:
                    engines[SPK_ENG].tensor_scalar(s6[:], m6[:], float(spk_thr), None,
                                                   Alu.is_gt)
                nc.sync.dma_start(out=spk_pr[:, 6 * r0:6 * (r0 + R)], in_=s6[:])
    return nc


def _build_const_nc(mem_row, spk_row):
    """Fast path: the 4 LUT rows are identical, so the output is constant.

    No input-dependent compute at all: fill SBUF with the constant rows and
    stream them out.  Pure output-bandwidth kernel.
    mem_row: [6] float32 values; spk_row: [6] ints (0/1).
    """
    import concourse.mybir as mybir
    from concourse import bacc
    from concourse.tile import TileContext

    f32 = mybir.dt.float32
    u8 = mybir.dt.uint8

    nc = bacc.Bacc(None, target_bir_lowering=False,
                   enable_partition_id=PART_ID)
    x = nc.dram_tensor("x", [SHARD, 2], f32, kind="ExternalInput")
    spk_o = nc.dram_tensor("spk", [SHARD, 6], u8, kind="ExternalOutput")
    mem_o = nc.dram_tensor("mem", [SHARD, 6], f32, kind="ExternalOutput")

    RT = SHARD // P                       # 256 rows per partition
    x_pr = x[:].rearrange("(p r) i -> p (r i)", p=P, r=RT)
    spk_pr = spk_o[:].rearrange("(p r) j -> p (r j)", p=P, r=RT)
    mem_pr = mem_o[:].rearrange("(p r) j -> p (r j)", p=P, r=RT)

    # small first piece (first DMA issues early); alternate the two hardware
    # DGE rings (sync/SP and scalar/ACT — ScalarE is otherwise idle here) so
    # the output drains on two queues with fast (~0.6us) first-byte latency
    # and no SWDGE teardown drain.
    PIECES = [32, 64, 80, 80]
    assert sum(PIECES) == RT
    p_starts = np.cumsum([0] + PIECES[:-1])
    piece_q = ["sync", "scalar", "sync", "scalar"]

    with TileContext(nc) as tc:
        with tc.tile_pool(name="pool", bufs=BUFS) as pool:
            # tiny read of x keeps the (dead) input bound in the NEFF
            xt = pool.tile([P, 2], f32, tag="x")
            nc.sync.dma_start(out=xt[:], in_=x_pr[:, 0:2])

            # one [P, 6] tile holding the constant mem row
            mc = pool.tile([P, 6], f32, tag="mc")
            for j in range(6):
                nc.vector.memset(mc[:, j:j + 1], float(mem_row[j]))

            # spikes: whole shard in one tile + one DMA.  Uniform row fills
            # via a uint32 view (4 bytes/elem); otherwise per-column strided.
            s6 = pool.tile([P, 6 * RT], u8, tag="s6")
            svals = [int(v) for v in spk_row]
            if len(set(svals)) == 1:
                s6_u32 = s6[:].bitcast(mybir.dt.uint32)
                nc.vector.memset(s6_u32, svals[0] * 0x01010101)
            else:
                s3 = s6[:].rearrange("p (r j) -> p r j", j=6)
                for j in range(6):
                    nc.vector.memset(s3[:, :, j], svals[j])
            nc.scalar.dma_start(out=spk_pr[:], in_=s6[:])

            # whole-shard mem tile: pieces are views, so no pool-slot
            # recycling ever gates a fill on an earlier DMA's completion
            m6 = pool.tile([P, 6 * RT], f32, tag="m6")
            for R, r0, q in zip(PIECES, p_starts, piece_q):
                mp = m6[:, 6 * r0:6 * (r0 + R)]
                m3 = mp.rearrange("p (r j) -> p r j", j=6)
                nc.vector.tensor_copy(
                    out=m3, in_=mc[:, None, :].to_broadcast([P, R, 6]))
                eng = nc.sync if q == "sync" else nc.scalar
                eng.dma_start(out=mem_pr[:, 6 * r0:6 * (r0 + R)], in_=mp)
    return nc


def _spk_threshold(info):
    """Pick the device-side spike threshold.

    If every LUT mem3 value is comfortably far from th_final, a uniform
    immediate threshold is safe (and fastest).  Otherwise use per-column
    midpoint thresholds, which are robust to any fp32 rounding.
    """
    if info["final_margin"] > 1e-3:
        return float(info["th_final"])
    cur3 = info["cur3"]          # [4, 6] float64
    th = info["th_final"]
    tau = np.empty(6)
    for j in range(6):
        vals = cur3[:, j]
        lo = vals[vals <= th]
        hi = vals[vals > th]
        if len(hi) == 0:
            tau[j] = 1e30
        elif len(lo) == 0:
            tau[j] = -1e30
        else:
            tau[j] = (lo.max() + hi.min()) / 2.0
    return tau


def _ensure_ntff_hook():
    """Some images lack antenv.axon_hooks; if BASS_TRACE is set without it,
    run_bass_kernel_spmd would crash on import.  Provide a working shim."""
    try:
        import antenv.axon_hooks  # noqa: F401
        return
    except ImportError:
        pass
    try:
        import sys
        import types
        import antenv
        hook = None
        try:
            from trn_agent_boot.trn_boot import _ntff_profile_via_ctypes
            hook = _ntff_profile_via_ctypes("/opt/axon/libaxon_pjrt.so")
        except Exception:
            hook = None
        mod = types.ModuleType("antenv.axon_hooks")
        mod.get_axon_ntff_profile_hook = lambda: hook
        mod.set_axon_ntff_profile_hook = lambda h: None
        sys.modules["antenv.axon_hooks"] = mod
        antenv.axon_hooks = mod
    except Exception:
        pass


def kernel(**inputs):
    global LAST_RESULT
    _ensure_ntff_hook()
    from concourse.bass_utils import run_bass_kernel_spmd

    data_in = np.ascontiguousarray(np.asarray(inputs["data_in"], np.float32))
    assert data_in.shape == (B_TOTAL, 2)

    spk_lut, mem_lut, info = _host_lut(inputs)
    c0, ca, cb, cab = _bilinear_coeffs(mem_lut)
    th0 = np.asarray(inputs["th0"], np.float64)
    spk_thr = _spk_threshold(info)

    # Fast path: if the 4 LUT rows coincide exactly (spikes die out in the
    # hidden layers) and every threshold margin is comfortable, the output is
    # a constant independent of data_in.
    cur3 = info["cur3"]
    const_out = (bool(np.all(cur3 == cur3[0]))
                 and min(info["hidden_margins"]) > 1e-4
                 and info["final_margin"] > 1e-3)
    if const_out:
        nc = _build_const_nc(mem_lut[0, 0].astype(np.float32),
                             spk_lut[0, 0].astype(np.int64))
    else:
        nc = _build_nc(float(th0[0]), float(th0[1]), c0, ca, cb, cab, spk_thr)
    nc.compile()

    shards = data_in.reshape(N_CORES, SHARD, 2)
    in_maps = [{"x": shards[i]} for i in range(N_CORES)]
    res = run_bass_kernel_spmd(nc, in_maps, core_ids=list(range(N_CORES)))
    LAST_RESULT = res

    spk = np.concatenate([np.asarray(r["spk"]) for r in res.results], axis=0)
    mem = np.concatenate([np.asarray(r["mem"]) for r in res.results], axis=0)
    return spk.astype(np.float32), mem.astype(np.float32, copy=False)
